# revision 7
# baseline (speedup 1.0000x reference)
"""Trainium2 Bass kernel for AU_GAZE_Affect7_LSTM_MLP.

Model: x [B=2048, 50, T=512] -> 4 branches (au 35ch, gaze 8, expr 5, va 2),
each a 3-layer bidirectional LSTM (hidden 8/dir), last-timestep feature
[B, 64] -> MLP 64->32(LN,ReLU)->16(LN,ReLU)->2.  Returns (logits [B,2],
h [B,1,64]).

Sharding: pure data parallel, batch 2048 -> 8 cores x 256 each.

Per-core design (feature-major: gates/hidden in partitions, batch in free):
  * per layer, two chains (fwd / bwd) step through time;
  * step: psum[128,256] = WxT.T @ x_t (+ WhhT.T @ h_{t-1}); gate rows
    [i(32) f(32) o(32) g(32)];
  * ACT: sigmoid(psum[0:96]+bias), tanh(psum[96:128]+bias);
  * DVE: m = i*tg; q = f*c; c' = m+q; h = o*tanh(c');
  * layer output sequences spill to DRAM ([T,32,B] per dir) and are the
    next layer's input windows;
  * layer-2 backward needs only t=T-1 => single step.
"""

import numpy as np

H = 8
BRANCH_DIMS = [("au", 35), ("gaze", 8), ("expr", 5), ("va", 2)]
BR_OFF = [0, 35, 43, 48]
NU = 4
GATE_REF_ROW = {0: 0, 1: 8, 2: 24, 3: 16}  # my gtype i,f,o,g -> torch row base

_CACHE = {}


def _np(a):
    return np.asarray(a, dtype=np.float32)


def _col(g, u, k):
    return g * 32 + u * 8 + k


def pack_weights(au_params, gaze_params, expr_params, va_params, mlp_params):
    branch_params = [au_params, gaze_params, expr_params, va_params]
    dirs = ["f", "b"]
    out = {}
    for l in range(3):
        din = 50 if l == 0 else 64
        wx_l = np.zeros((2, din, 128), np.float32)
        whh_l = np.zeros((2, 32, 128), np.float32)
        b_l = np.zeros((2, 128), np.float32)
        for d, tag in enumerate(dirs):
            for u in range(NU):
                p = branch_params[u][l]
                wih = _np(p["wih_" + tag])
                whh_u = _np(p["whh_" + tag])
                bsum = _np(p["bih_" + tag]) + _np(p["bhh_" + tag])
                for g in range(4):
                    rb = GATE_REF_ROW[g]
                    for k in range(H):
                        c = _col(g, u, k)
                        r = rb + k
                        if l == 0:
                            off, dd = BR_OFF[u], BRANCH_DIMS[u][1]
                            wx_l[d, off:off + dd, c] = wih[r, :]
                        else:
                            wx_l[d, u * 8:(u + 1) * 8, c] = wih[r, 0:8]
                            wx_l[d, 32 + u * 8:32 + (u + 1) * 8, c] = wih[r, 8:16]
                        whh_l[d, u * 8:(u + 1) * 8, c] = whh_u[r, :]
                        b_l[d, c] = bsum[r]
        out[f"wx{l}"] = wx_l
        out[f"whh{l}"] = whh_l
        out[f"bias{l}"] = b_l

    m = mlp_params
    ref_j = np.zeros(64, np.int64)
    for d in range(2):
        for u in range(NU):
            for k in range(8):
                ref_j[d * 32 + u * 8 + k] = u * 16 + d * 8 + k
    perm = np.zeros((64, 64), np.float32)
    for r in range(64):
        perm[r, ref_j[r]] = 1.0
    w1 = _np(m["l1"]["w"])
    w1m = np.zeros((64, 32), np.float32)
    for r in range(64):
        w1m[r, :] = w1[:, ref_j[r]]

    def rep(v):
        v = _np(v)
        return np.repeat(v[None, :], 128, axis=0).copy()

    out.update({
        "perm": perm, "w1m": w1m,
        "w2m": _np(m["l2"]["w"]).T.copy(), "w3m": _np(m["l3"]["w"]).T.copy(),
        "b1r": rep(m["l1"]["b"]), "g1r": rep(m["ln1_g"]), "e1r": rep(m["ln1_b"]),
        "b2r": rep(m["l2"]["b"]), "g2r": rep(m["ln2_g"]), "e2r": rep(m["ln2_b"]),
        "b3r": rep(m["l3"]["b"]),
        "ident": np.eye(128, dtype=np.float32),
    })
    return out


def build_program(B=256, T=512, TC=8):
    import concourse.bass as bass  # noqa: F401
    import concourse.mybir as mybir
    import concourse.tile as tile
    from concourse import bacc

    F32 = mybir.dt.float32
    AF = mybir.ActivationFunctionType
    OP = mybir.AluOpType
    AX = mybir.AxisListType
    NCH = T // TC
    assert T % TC == 0

    nc = bacc.Bacc("TRN2", target_bir_lowering=False, debug=False)

    X_d = nc.dram_tensor("x", [B, 50, T], F32, kind="ExternalInput")
    wdecl = {
        "wx0": [2, 50, 128], "wx1": [2, 64, 128], "wx2": [2, 64, 128],
        "whh0": [2, 32, 128], "whh1": [2, 32, 128], "whh2": [2, 32, 128],
        "bias0": [2, 128], "bias1": [2, 128], "bias2": [2, 128],
        "perm": [64, 64], "w1m": [64, 32], "w2m": [32, 16], "w3m": [16, 2],
        "b1r": [128, 32], "g1r": [128, 32], "e1r": [128, 32],
        "b2r": [128, 16], "g2r": [128, 16], "e2r": [128, 16],
        "b3r": [128, 2], "ident": [128, 128],
    }
    wd = {k: nc.dram_tensor(k, s, F32, kind="ExternalInput")
          for k, s in wdecl.items()}
    LG_d = nc.dram_tensor("logits", [B, 2], F32, kind="ExternalOutput")
    HO_d = nc.dram_tensor("hout", [B, 64], F32, kind="ExternalOutput")

    with tile.TileContext(nc) as tc:
        with tc.tile_pool(name="const", bufs=1) as cpool, \
             tc.tile_pool(name="state", bufs=1) as spool, \
             tc.tile_pool(name="inwin", bufs=2) as ipool, \
             tc.tile_pool(name="outwin", bufs=2) as opool, \
             tc.tile_pool(name="act", bufs=3) as apool, \
             tc.tile_pool(name="gates", bufs=3, space="PSUM") as gpool, \
             tc.tile_pool(name="mlppsum", bufs=2, space="PSUM") as mpool, \
             tc.tile_pool(name="dram", bufs=2, space="DRAM") as dpool:

            wx_sb, whh_sb, bias_sb = {}, {}, {}
            for l in range(3):
                din = 50 if l == 0 else 64
                for d in range(2):
                    t = cpool.tile([din, 128], F32, tag=f"wx{l}{d}")
                    nc.sync.dma_start(t[:, :], wd[f"wx{l}"][d, :, :])
                    wx_sb[(l, d)] = t
                    t = cpool.tile([32, 128], F32, tag=f"whh{l}{d}")
                    nc.sync.dma_start(t[:, :], wd[f"whh{l}"][d, :, :])
                    whh_sb[(l, d)] = t
                    t = cpool.tile([128, 1], F32, tag=f"bias{l}{d}")
                    nc.sync.dma_start(
                        t[:, :],
                        wd[f"bias{l}"][d, :].rearrange("(p o) -> p o", o=1))
                    bias_sb[(l, d)] = t
            msb = {}
            for k in ["perm", "w1m", "w2m", "w3m", "b1r", "g1r", "e1r",
                      "b2r", "g2r", "e2r", "b3r", "ident"]:
                t = cpool.tile(wdecl[k], F32, tag=k)
                nc.sync.dma_start(t[:, :], wd[k][:, :])
                msb[k] = t

            def load_in_chunk(l, d, cc, seq_r):
                t0 = cc * TC if d == 0 else T - TC * (cc + 1)
                if l == 0:
                    # free layout (b, t) so the DMA APs balance to <=3 dims
                    w = ipool.tile([50, TC * B], F32, tag=f"in{d}")
                    nc.sync.dma_start(
                        w[:, :].rearrange("c (b t) -> c b t", t=TC),
                        X_d[:, :, t0:t0 + TC].rearrange("b c t -> c b t"))
                else:
                    w = ipool.tile([64, TC * B], F32, tag=f"in{d}")
                    for sd in range(2):
                        nc.sync.dma_start(
                            w[sd * 32:(sd + 1) * 32, :]
                            .rearrange("r (t b) -> r t b", t=TC),
                            seq_r[sd][t0:t0 + TC, :, :]
                            .rearrange("t r b -> r t b"))
                return w

            def flush_out_chunk(l, d, w, cc, seq_w):
                if seq_w is None:
                    return
                t0 = cc * TC if d == 0 else T - TC * (cc + 1)
                nc.sync.dma_start(
                    seq_w[d][t0:t0 + TC, :, :].rearrange("t r b -> r t b"),
                    w[:, :].rearrange("r (t b) -> r t b", t=TC))

            def lstm_step(l, d, in_w, tl, out_w, col_out,
                          h_prev, c_prev, c_next):
                g = gpool.tile([128, B], F32, tag=f"g{d}")
                first = h_prev is None
                if l == 0:
                    mv = in_w[:, :].rearrange("c (b t) -> c t b", t=TC)[:, tl, :]
                else:
                    mv = in_w[:, tl * B:(tl + 1) * B]
                nc.tensor.matmul(g[:, :], wx_sb[(l, d)][:, :], mv,
                                 start=True, stop=first)
                if not first:
                    nc.tensor.matmul(g[:, :], whh_sb[(l, d)][:, :], h_prev,
                                     start=False, stop=True)
                S = apool.tile([96, B], F32, tag=f"S{d}")
                nc.scalar.activation(S[:, :], g[0:96, :], AF.Sigmoid,
                                     bias=bias_sb[(l, d)][0:96, :])
                TG = apool.tile([32, B], F32, tag=f"TG{d}")
                nc.scalar.activation(TG[:, :], g[96:128, :], AF.Tanh,
                                     bias=bias_sb[(l, d)][96:128, :])
                m = apool.tile([32, B], F32, tag=f"m{d}")
                nc.vector.tensor_tensor(m[:, :], S[0:32, :], TG[:, :], OP.mult)
                cn = c_next[32:64, :]
                if first:
                    nc.vector.tensor_copy(cn, m[:, :])
                else:
                    q = apool.tile([32, B], F32, tag=f"q{d}")
                    nc.vector.tensor_tensor(q[:, :], S[32:64, :], c_prev,
                                            OP.mult)
                    nc.vector.tensor_tensor(cn, m[:, :], q[:, :], OP.add)
                TCs = apool.tile([96, B], F32, tag=f"TCs{d}")
                nc.scalar.activation(TCs[64:96, :], cn, AF.Tanh)
                h = out_w[:, col_out:col_out + B]
                nc.vector.tensor_tensor(h, S[64:96, :], TCs[64:96, :], OP.mult)
                return h, cn

            fin_fwd = None
            fin_bwd = None
            seq_r = None  # {d: dram tile} written by previous layer
            for l in range(3):
                seq_w = None
                if l < 2:
                    seq_w = {}
                    for d in range(2):
                        seq_w[d] = dpool.tile([T, 32, B], F32,
                                              tag=f"seq{d}", name=f"seqt{l}{d}")
                st = {d: {"h": None, "c": None,
                          "cA": spool.tile([64, B], F32, tag=f"cA{l}{d}", name=f"cA{l}{d}"),
                          "cB": spool.tile([64, B], F32, tag=f"cB{l}{d}", name=f"cB{l}{d}"),
                          "in": None, "out": None}
                      for d in range(2)}
                dirs_l = (0,) if l == 2 else (0, 1)
                for s in range(T):
                    for d in dirs_l:
                        stx = st[d]
                        if s % TC == 0:
                            cc = s // TC
                            if cc > 0:
                                flush_out_chunk(l, d, stx["out"], cc - 1,
                                                seq_w)
                            stx["in"] = load_in_chunk(l, d, cc, seq_r)
                            ow = opool.tile([32, TC * B], F32, tag=f"out{d}")
                            stx["out"] = ow
                        tl = s % TC if d == 0 else TC - 1 - (s % TC)
                        c_next = stx["cA"] if (s % 2 == 0) else stx["cB"]
                        h, c = lstm_step(l, d, stx["in"], tl,
                                         stx["out"], tl * B,
                                         stx["h"], stx["c"], c_next)
                        stx["h"], stx["c"] = h, c
                for d in dirs_l:
                    flush_out_chunk(l, d, st[d]["out"], NCH - 1, seq_w)
                if l == 2:
                    fin_fwd = st[0]["h"]
                    bw_out = opool.tile([32, B], F32, tag="bw2")
                    h, _ = lstm_step(2, 1, st[0]["in"], TC - 1,
                                     bw_out, 0, None, None, st[1]["cA"])
                    fin_bwd = h
                seq_r = seq_w

            # ---- MLP head ----
            F = spool.tile([64, B], F32, tag="feat")
            nc.vector.tensor_copy(F[0:32, :], fin_fwd)
            nc.vector.tensor_copy(F[32:64, :], fin_bwd)

            def layer_norm_relu(z, nf, g_t, e_t):
                ssum = apool.tile([128, 1], F32, tag="ssum")
                nc.vector.tensor_reduce(ssum[:, :], z[:, :], AX.X, OP.add)
                nm = apool.tile([128, 1], F32, tag="nm")
                nc.vector.tensor_scalar_mul(nm[:, :], ssum[:, :], -1.0 / nf)
                zc = apool.tile([128, nf], F32, tag="zc")
                nc.scalar.activation(zc[:, :], z[:, :], AF.Identity,
                                     bias=nm[:, :])
                sq = apool.tile([128, nf], F32, tag="sq")
                ss = apool.tile([128, 1], F32, tag="ss")
                nc.scalar.activation(sq[:, :], zc[:, :], AF.Square,
                                     accum_out=ss[:, :])
                ve = apool.tile([128, 1], F32, tag="ve")
                nc.vector.tensor_scalar(ve[:, :], ss[:, :], 1.0 / nf, 1e-6,
                                        OP.mult, OP.add)
                sd = apool.tile([128, 1], F32, tag="sd")
                nc.scalar.activation(sd[:, :], ve[:, :], AF.Sqrt)
                rr = apool.tile([128, 1], F32, tag="rr")
                nc.vector.reciprocal(rr[:, :], sd[:, :])
                zn = apool.tile([128, nf], F32, tag="zn")
                nc.scalar.mul(zn[:, :], zc[:, :], rr[:, :])
                y = apool.tile([128, nf], F32, tag="y")
                nc.vector.tensor_tensor(y[:, :], zn[:, :], g_t[:, :], OP.mult)
                y2 = apool.tile([128, nf], F32, tag="yb")
                nc.vector.tensor_tensor(y2[:, :], y[:, :], e_t[:, :], OP.add)
                y3 = apool.tile([128, nf], F32, tag="yr")
                nc.vector.tensor_scalar_max(y3[:, :], y2[:, :], 0.0)
                return y3

            for half in range(B // 128):
                Fh = F[:, half * 128:(half + 1) * 128]
                ph = mpool.tile([128, 64], F32, tag="mlp")
                nc.tensor.matmul(ph[:, :], Fh, msb["perm"][:, :],
                                 start=True, stop=True)
                hT = apool.tile([128, 64], F32, tag="hT")
                nc.vector.tensor_copy(hT[:, :], ph[:, :])
                nc.sync.dma_start(HO_d[half * 128:(half + 1) * 128, :],
                                  hT[:, :])
                p1 = mpool.tile([128, 32], F32, tag="mlp")
                nc.tensor.matmul(p1[:, :], Fh, msb["w1m"][:, :],
                                 start=True, stop=True)
                z1 = apool.tile([128, 32], F32, tag="z1")
                nc.vector.tensor_tensor(z1[:, :], p1[:, :], msb["b1r"][:, :],
                                        OP.add)
                y1 = layer_norm_relu(z1, 32, msb["g1r"], msb["e1r"])
                pt1 = mpool.tile([32, 128], F32, tag="mlp")
                nc.tensor.transpose(pt1[:, :], y1[:, :], msb["ident"][:, :])
                y1t = apool.tile([32, 128], F32, tag="y1t")
                nc.vector.tensor_copy(y1t[:, :], pt1[:, :])
                p2 = mpool.tile([128, 16], F32, tag="mlp")
                nc.tensor.matmul(p2[:, :], y1t[:, :], msb["w2m"][:, :],
                                 start=True, stop=True)
                z2 = apool.tile([128, 16], F32, tag="z2")
                nc.vector.tensor_tensor(z2[:, :], p2[:, :], msb["b2r"][:, :],
                                        OP.add)
                y2 = layer_norm_relu(z2, 16, msb["g2r"], msb["e2r"])
                pt2 = mpool.tile([16, 128], F32, tag="mlp")
                nc.tensor.transpose(pt2[:, :], y2[:, :], msb["ident"][:, :])
                y2t = apool.tile([16, 128], F32, tag="y2t")
                nc.vector.tensor_copy(y2t[:, :], pt2[:, :])
                p3 = mpool.tile([128, 2], F32, tag="mlp")
                nc.tensor.matmul(p3[:, :], y2t[:, :], msb["w3m"][:, :],
                                 start=True, stop=True)
                lg = apool.tile([128, 2], F32, tag="lg")
                nc.vector.tensor_tensor(lg[:, :], p3[:, :], msb["b3r"][:, :],
                                        OP.add)
                nc.sync.dma_start(LG_d[half * 128:(half + 1) * 128, :],
                                  lg[:, :])

    nc.compile()
    return nc


def kernel(x, au_params, gaze_params, expr_params, va_params, mlp_params):
    from concourse import bass_utils

    x = _np(x)
    B_full, C, T = x.shape
    n_cores = 8
    Bc = B_full // n_cores

    key = (Bc, T)
    if key not in _CACHE:
        _CACHE[key] = build_program(B=Bc, T=T)
    nc = _CACHE[key]

    w = pack_weights(au_params, gaze_params, expr_params, va_params,
                     mlp_params)
    in_maps = []
    for c in range(n_cores):
        m = {"x": np.ascontiguousarray(x[c * Bc:(c + 1) * Bc])}
        m.update(w)
        in_maps.append(m)

    res = bass_utils.run_bass_kernel_spmd(nc, in_maps,
                                          core_ids=list(range(n_cores)))
    logits = np.concatenate([r["logits"] for r in res.results], axis=0)
    hout = np.concatenate([r["hout"] for r in res.results], axis=0)
    return logits, hout[:, None, :]
